# revision 27
# baseline (speedup 1.0000x reference)
"""Trainium2 Bass kernel for nn_Net_Deform (deformable-conv CNN).

Self-contained: shards the batch over 8 NeuronCores (pure data parallel),
runs a Bass/Tile program per core, gathers full output.

v2 layout notes (S=256 samples/core, s = sq*64 + s64):
 F1 (XH): [128 p=(y+2)*4+sq, 2048 c=(x+2)*64+s64], y,x in [-2,30),
     replicate-padded on host, fp16.
 D1X: deformed1 with zero pads, same indexing as F1. fp16.
 conv1: single shared lhsT [120,128], K=(d5, ryi6, sq4) rel to y-pair t,
     via im2col IC_t [120, 1792] built with 5 DMAs/t from D1X.
     M = par*64 + sq*16 + co.
 h1 (F2): [128 p=(yy+1)*8+ci8, 7168 c=sq*1792 + (ci//8)*896 + xx*64 + s64]
     pad rows yy=-1/14 replicated. Scatter via DRAM bounce (2 DMAs/t).
 off2 psums/monos/deform2 tiles: psum-order cols lx*256 + (sq*64+s64).
 ABC (conv2 K relayout): [128 p = (ci//8)*64 + ai*8 + (ci%8), 3584 c =
     xi*256 + s], ai = yi - y0, bases A=0 B=2 C=6; filled with ONE plain
     64-partition DMA per (quarter, alloc).
 conv2 M = par*64 + cc*2 + yh; fc as baseline.
All lhsT consts packed into one [128, NC] fp16 dram tensor (1 DMA) +
one [64, 4] fp32 bias pack.
"""
import sys
sys.path.insert(0, '/opt/trn_rl_repo')
import numpy as np
from contextlib import ExitStack
import concourse.bass as bass
import concourse.mybir as mybir
import concourse.bacc as bacc
import concourse.tile as tile
from concourse.ap import AP

S = 256
SQ, S64 = 4, 64
NCORES = 8

def f1p(y, sq): return (y + 2) * 4 + sq
def f1c(x, s64=0): return (x + 2) * 64 + s64

_YY = np.clip(np.arange(-2, 30), 0, 27)
def host_prep_x(x_core):
    """(S,1,28,28) -> F1 [128, 2048] replicate-padded."""
    img = x_core.reshape(S, 28, 28)
    big = img[:, _YY][:, :, _YY]                      # (S, 32, 32)
    b = big.reshape(SQ, S64, 32, 32)
    out = b.transpose(2, 0, 3, 1).reshape(128, 2048)  # (y,sq,x,s64)
    return np.ascontiguousarray(out.astype(np.float32))

# ---------------- off1 schedule (F1 -> F1-order psum) ----------------
def off1_shuffle(y, x, k):
    j = 56 * y + 2 * x + k
    c = j // 784
    rem = j % 784
    return c, rem // 28, rem % 28

def build_off1_sched(off1_w):
    W = np.asarray(off1_w, np.float32)
    ent = []
    for k in (0, 1):
        for h in (0, 1):
            for dxt in (0, 1, 2):
                L = np.zeros((128, 128), np.float32)
                nz = False
                for y in range(28):
                    c, yo, xo0 = off1_shuffle(y, h * 14, k)
                    for sq in range(SQ):
                        m = f1p(y, sq)
                        for dyt in (0, 1, 2):
                            yi = yo + dyt - 1
                            if 0 <= yi < 28:
                                L[f1p(yi, sq), m] += W[c, 0, dyt, dxt]
                                nz = True
                if nz:
                    ent.append(('offy' if k == 0 else 'offx', k, h, dxt, L))
    return ent

def off1_rhs_cols(k, h, dxt):
    xs, xis = [], []
    for x in range(h * 14, h * 14 + 14):
        xi = 2 * x + k - 28 * h + dxt - 1
        if 0 <= xi < 28:
            xs.append(x); xis.append(xi)
    return xs, xis

# ---------------- conv1 shared lhsT (im2col K = (d, ryi, sq)) ------------
def build_conv1_lhsT_shared(conv1_w, ry_lo=-2, ry_hi=3):
    """zero out K rows with ryi outside [ry_lo, ry_hi] (edge pads)."""
    W = np.asarray(conv1_w, np.float32)
    L = np.zeros((120, 128), np.float32)
    for d in range(5):
        for ryi in range(-2, 4):
            if not (ry_lo <= ryi <= ry_hi):
                continue
            for sq in range(4):
                k = d * 24 + (ryi + 2) * 4 + sq
                for par in range(2):
                    dy = ryi - par
                    if not (-2 <= dy <= 2):
                        continue
                    for co in range(16):
                        L[k, par * 64 + sq * 16 + co] = W[co, 0, dy + 2, d]
    return L

# ---------------- off2 schedule ----------------
def off2_shuffle(c, y, x, k):
    j = 392 * c + 28 * y + 2 * x + k
    ch = j // 196
    rem = j % 196
    return ch, rem // 14, rem % 14

def f2p(yy, ci): return (yy + 1) * 8 + (ci % 8)

def build_off2_sched(off2_w):
    W = np.asarray(off2_w, np.float32)
    ent = []
    for k in (0, 1):
        for gout in (0, 1):
            for h in (0, 1):
                for dxt in (0, 1, 2):
                    for gin in (0, 1):
                        L = np.zeros((128, 128), np.float32)
                        nz = False
                        for cc in range(gout * 8, gout * 8 + 8):
                            for y in range(14):
                                ch, yo, xo = off2_shuffle(cc, y, h * 7, k)
                                for dyt in (0, 1, 2):
                                    yi = yo + dyt - 1
                                    if not (0 <= yi < 14):
                                        continue
                                    for ci in range(gin * 8, gin * 8 + 8):
                                        L[f2p(yi, ci), f2p(y, cc)] += W[ch, ci, dyt, dxt]
                                        nz = True
                        if nz:
                            ent.append(('offy' if k == 0 else 'offx', k, gout, h, dxt, gin, L))
    return ent

def off2_rhs_cols(k, h, dxt):
    xs, xis = [], []
    for x in range(h * 7, h * 7 + 7):
        xi = 2 * x + k - 14 * h + dxt - 1
        if 0 <= xi < 14:
            xs.append(x); xis.append(xi)
    return xs, xis

# ---------------- conv2 schedule (ABC new K order) ----------------
CONV2_BASE = {0: ('A', 0), 1: ('B', 2), 2: ('C', 6), 3: ('C', 6)}
def abc_row(ci, ai):
    return (ci // 8) * 64 + ai * 8 + (ci % 8)

def build_conv2_lhsT(conv2_w):
    """{(ygr, dx): (al, lhsT[128,128])}; K row = abc_row(ci, yi - y0)."""
    W = np.asarray(conv2_w, np.float32)
    out = {}
    for ygr in range(4):
        al, y0 = CONV2_BASE[ygr]
        for dx in range(-2, 3):
            L = np.zeros((128, 128), np.float32)
            nz = False
            for par in range(2):
                for yh in range(2):
                    yp = ygr * 4 + 2 * yh + par
                    if yp >= 14:
                        continue
                    for cc in range(32):
                        m = par * 64 + cc * 2 + yh
                        for dy in range(-2, 3):
                            yi = yp + dy
                            if not (0 <= yi < 14):
                                continue
                            ai = yi - y0
                            assert 0 <= ai < 8
                            for ci in range(16):
                                L[abc_row(ci, ai), m] += W[cc, ci, dy + 2, dx + 2]
                                nz = True
            if nz:
                out[(ygr, dx)] = (al, L)
    return out

def build_fc_lhsT(fc_w):
    W = np.asarray(fc_w, np.float32)
    out = {}
    for ygr in range(4):
        for xq in range(7):
            L = np.zeros((64, 16), np.float32)
            for cc in range(32):
                for yh in range(2):
                    ypp = ygr * 2 + yh
                    if ypp >= 7:
                        continue
                    L[cc * 2 + yh, :10] = W[:, cc * 49 + ypp * 7 + xq]
            out[(ygr, xq)] = L
    return out


F32, F16 = mybir.dt.float32, mybir.dt.float16
AF = mybir.ActivationFunctionType
ALU = mybir.AluOpType
TP = dict(tile_position=(0, 0))


class ConstPack:
    """Packs fp16 [rows<=128, cols] consts into one [128, NC] tensor."""
    def __init__(self):
        self.mats = []      # list of np arrays (rows, cols) fp16
        self.index = {}     # key -> (col0, rows, cols)
        self.cols = 0

    def add(self, arr):
        a = np.ascontiguousarray(np.asarray(arr, np.float16))
        key = (a.shape, a.tobytes())
        if key not in self.index:
            self.index[key] = (self.cols, a.shape[0], a.shape[1])
            self.mats.append(a)
            self.cols += a.shape[1]
        return self.index[key]

    def host_array(self):
        out = np.zeros((128, self.cols), np.float16)
        c = 0
        for a in self.mats:
            out[0:a.shape[0], c:c + a.shape[1]] = a
            c += a.shape[1]
        return out


def build_program(w):
    # ---------- host-side schedules ----------
    off1 = build_off1_sched(w['off1_w'])
    c1L = {t: build_conv1_lhsT_shared(
        w['conv1_w'],
        ry_lo=(0 if t == 0 else -2),
        ry_hi=(1 if t == 13 else 3)) for t in range(14)}
    off2 = build_off2_sched(w['off2_w'])
    c2 = build_conv2_lhsT(w['conv2_w'])
    fcL = build_fc_lhsT(w['fc_w'])

    packA = ConstPack()   # phase-1: off1 + conv1 (freed after phase 1)
    packB = ConstPack()   # phase-2: off2 + conv2 + fc (persistent)
    off1_ref = [(im, k, h, dxt, packA.add(L)) for (im, k, h, dxt, L) in off1]
    c1_ref = {t: packA.add(c1L[t]) for t in range(14)}
    off2_ref = [(im, k, go, h, dxt, gin, packB.add(L))
                for (im, k, go, h, dxt, gin, L) in off2]
    c2_ref = {key: (al, packB.add(L)) for key, (al, L) in c2.items()}
    fc_ref = {key: packB.add(L) for key, L in fcL.items()}

    bias = np.zeros((64, 4), np.float32)
    bias[:, 0] = np.asarray(w['conv1_b'], np.float32)[np.arange(64) % 16]
    bias[:, 1] = np.asarray(w['conv2_b'], np.float32)[np.arange(64) // 2]
    bias[0:10, 2] = np.asarray(w['fc_b'], np.float32)

    consts_np = {'CPA': packA.host_array(), 'CPB': packB.host_array(),
                 'BIA': bias}

    nc = bacc.Bacc("TRN2", target_bir_lowering=False, debug=False)
    XH_d = nc.dram_tensor("XH", [128, 2048], F16, kind="ExternalInput").ap()
    CPA_d = nc.dram_tensor("CPA", [128, packA.cols], F16, kind="ExternalInput").ap()
    CPB_d = nc.dram_tensor("CPB", [128, packB.cols], F16, kind="ExternalInput").ap()
    BIA_d = nc.dram_tensor("BIA", [64, 4], F32, kind="ExternalInput").ap()
    SCR_d = nc.dram_tensor("SCR", [14, 64, 896], F16, kind="Internal").ap()
    OUT_d = nc.dram_tensor("OUT", [16, 256], F32, kind="ExternalOutput").ap()

    with ExitStack() as ctx:
        tc = ctx.enter_context(tile.TileContext(nc))
        V, SC, GP = nc.vector, nc.scalar, nc.gpsimd
        MM = nc.tensor.matmul

        CPBs = nc.alloc_sbuf_tensor("CPBs", [128, packB.cols], F16).ap()
        nc.sync.dma_start(CPBs, CPB_d)
        BIAs = nc.alloc_sbuf_tensor("BIAs", [64, 4], F32).ap()
        nc.scalar.dma_start(BIAs, BIA_d)

        CPAs = [None]

        def crefA(ref):
            c0, rows, cols = ref
            return CPAs[0][0:rows, c0:c0 + cols]

        def cref(ref):
            c0, rows, cols = ref
            return CPBs[0:rows, c0:c0 + cols]

        b64 = BIAs[0:64, 0:1]
        b2c = BIAs[0:64, 1:2]
        fcb = BIAs[0:16, 2:3]

        def emit_banked(banks):
            for bk in sorted(banks):
                lst = sorted(banks[bk], key=lambda e: e[0])
                for i, (key, out, Lap, rhs) in enumerate(lst):
                    MM(out, Lap, rhs, start=(i == 0), stop=(i == len(lst) - 1), **TP)

        spool = ctx.enter_context(tc.tile_pool(name="stg", bufs=2))

        # persistent
        h1 = nc.alloc_sbuf_tensor("h1", [128, 7168], F16).ap()

        # ================= PHASE 1 =================
        with tc.tile_pool(name="ph1", bufs=1) as p1, \
             tc.tile_pool(name="ps1", bufs=2, space="PSUM") as ppool:
            XH = p1.tile([128, 2048], F16, tag="XH", name="XH")
            nc.sync.dma_start(XH, XH_d)
            CPAt = p1.tile([128, packA.cols], F16, tag="CPA", name="CPAt")
            CPAs[0] = CPAt
            csplit = min(1664, packA.cols)
            nc.sync.dma_start(CPAt[:, 0:csplit], CPA_d[:, 0:csplit])
            if packA.cols > csplit:
                nc.scalar.dma_start(CPAt[:, csplit:], CPA_d[:, csplit:])

            def img1v(name):
                return p1.tile([128, 1792], F16, tag=name, name=name)

            # ---- dx tensors + shifted operands first (independent of off1) ----
            dxp, dxm = img1v("dxp"), img1v("dxm")
            XV = XH[:, 128:1920]
            V.tensor_sub(dxp, XH[:, f1c(1):f1c(29)], XV)
            V.tensor_sub(dxm, XH[:, f1c(-1):f1c(27)], XV)
            shifted = {}
            for base, nm in ((XV, 'X'), (dxp, 'dxp'), (dxm, 'dxm')):
                for dy, sfx in ((1, 'p4'), (-1, 'm4')):
                    tgt = img1v(nm + sfx)
                    q = nc.sync
                    if dy == 1:
                        q.dma_start(tgt[0:124], base[4:128])
                        q.dma_start(tgt[124:128], base[124:128])
                    else:
                        q.dma_start(tgt[4:128], base[0:124])
                        q.dma_start(tgt[0:4], base[0:4])
                    shifted[(nm, sfx)] = tgt

            # ---- off1 matmuls ----
            ps_off = {}
            for im in ('offy', 'offx'):
                ps_off[im] = ppool.tile([128, 2048], F32, tag="big", name=f"ps_{im}")
            acc = {(im, x): [] for im in ('offy', 'offx') for x in range(28)}
            for (im, k, h, dxt, ref) in off1_ref:
                Lap = crefA(ref)
                xs, xis = off1_rhs_cols(k, h, dxt)
                runs = []
                for xo_, xi_ in zip(xs, xis):
                    if runs and runs[-1][-1][0] == xo_ - 1 and \
                       (xo_ % 8) != 0 and len(runs[-1]) < 8:
                        runs[-1].append((xo_, xi_))
                    else:
                        runs.append([(xo_, xi_)])
                for r in runs:
                    x0, xi0, n = r[0][0], r[0][1], len(r)
                    rhs = XH.rearrange("p (x s) -> p x s", s=64)[:, xi0 + 2:xi0 + 2 + 2 * n]
                    rhs = rhs.rearrange("p (n two) s -> p two n s", two=2)[:, 0]
                    acc[(im, x0)].append((ps_off[im].rearrange("p (x s) -> p x s", s=64)[:, x0:x0 + n],
                                          rhs, Lap))
            banks = {}
            for im in ('offy', 'offx'):
                for x in range(28):
                    for (out, rhs, Lap) in acc[(im, x)]:
                        banks.setdefault((im, x // 8), []).append(
                            ((x, -(x + out.shape[1])), out, Lap, rhs))
            emit_banked(banks)

            # ---- monos ----
            up1, um1 = img1v("up1"), img1v("um1")
            vp1, vm1 = img1v("vp1"), img1v("vm1")
            SC.activation(up1, ps_off['offy'][:, 0:1792], AF.Relu)
            V.tensor_scalar(um1, ps_off['offy'][:, 0:1792], -1.0, 0.0, ALU.mult, ALU.max)
            SC.activation(vp1, ps_off['offx'][:, 0:1792], AF.Relu)
            V.tensor_scalar(vm1, ps_off['offx'][:, 0:1792], -1.0, 0.0, ALU.mult, ALU.max)

            # ---- deform1 (all vector) ----
            S0, e1, e2, e3 = img1v("S0"), img1v("e1"), img1v("e2"), img1v("e3")
            D1X = p1.tile([128, 2048], F16, tag="D1X", name="D1X")
            GP.memset(D1X[:, 0:128], 0.0)
            GP.memset(D1X[:, 1920:2048], 0.0)
            V.tensor_mul(e1, vp1, dxp)
            V.tensor_mul(e2, vm1, dxm)
            V.tensor_add(S0, XV, e1)
            V.tensor_add(S0, S0, e2)
            for sfx, mono, dst in (('p4', up1, e1), ('m4', um1, e3)):
                V.tensor_mul(e2, vp1, shifted[('dxp', sfx)])
                V.tensor_add(dst, shifted[('X', sfx)], e2)
                V.tensor_mul(e2, vm1, shifted[('dxm', sfx)])
                V.tensor_add(dst, dst, e2)
                V.tensor_sub(dst, dst, S0)
                V.tensor_mul(dst, mono, dst)
            V.tensor_add(e1, e1, e3)
            V.tensor_add(D1X[:, 128:1920], S0, e1)

            # ---- conv1 via im2col ----
            icq = [nc.scalar, nc.gpsimd, nc.gpsimd, nc.scalar, nc.gpsimd]
            for t in range(14):
                C1 = crefA(c1_ref[t])
                IC = p1.tile([120, 1792], F16, tag="IC", bufs=3, name=f"IC_{t}")
                for d in range(5):
                    icq[(t * 5 + d) % 5].dma_start(
                        IC[d * 24:(d + 1) * 24],
                        D1X[8 * t:8 * t + 24, d * 64:d * 64 + 1792])
                psum = ppool.tile([128, 2048], F32, tag="big", name=f"psc1_{t}")
                for b in range(4):
                    ncol = 512 if b < 3 else 256
                    MM(psum[:, b * 512:b * 512 + ncol], C1,
                       IC[0:120, b * 512:b * 512 + ncol], start=True, stop=True, **TP)
                pv = psum.rearrange("p (xh two s) -> p xh two s", two=2, s=64)
                pxa = spool.tile([128, 896], F32, tag="pxa", bufs=2, name=f"pxa_{t}")
                V.tensor_copy(pxa.rearrange("p (x s) -> p x s", s=64), pv[:, 0:14, 0])
                px = spool.tile([128, 896], F16, tag="px", name=f"px_{t}")
                V.tensor_max(px.rearrange("p (x s) -> p x s", s=64),
                             pxa.rearrange("p (x s) -> p x s", s=64), pv[:, 0:14, 1])
                pxs = spool.tile([64, 896], F16, tag="pxs", name=f"pxs_{t}")
                nc.sync.dma_start(pxs, px[64:128])
                py = spool.tile([64, 896], F16, tag="py", name=f"py_{t}")
                V.tensor_max(py, px[0:64], pxs)
                stg = spool.tile([64, 896], F16, tag="stgc1", name=f"stg_{t}")
                V.tensor_scalar(stg, py, b64, 0.0, ALU.add, ALU.max)
                # bounce through DRAM, then ONE coalesced scatter into h1
                # h1 col = sq*1792 + gp*896 + x*64 + s; src (co, sq, gp, c):
                # sq/gp merge on src (14336 = 2*7168), (gp, c) stay split on dst
                nc.sync.dma_start(SCR_d[t], stg)
                dsts = [(t + 1) * 8]
                if t == 0:
                    dsts.append(0)
                if t == 13:
                    dsts.append(120)
                for p0 in dsts:
                    src = AP(SCR_d.tensor, SCR_d.offset + t * 64 * 896,
                             [[896, 8], [7168, 8], [1, 896]])
                    dst = AP(h1.tensor, h1.offset + p0 * 7168,
                             [[7168, 8], [896, 8], [1, 896]])
                    nc.sync.dma_start(dst, src)

        # ================= PHASE 2 =================
        with tc.tile_pool(name="ph2", bufs=1) as p2:
            h1v = h1.rearrange("p (sq g x s) -> p sq g x s", sq=4, g=2, s=64)

            def hviewH(g, h):
                # sq-major half view of h1 g-block: (sq: 1792, 4)(c: 448)
                return h1.rearrange("p (sq g xh c) -> p g xh sq c",
                                    sq=4, g=2, xh=2)[:, g, h]

            def shviewH(tl, h):
                # sq-major half view of a g-block tile [128, 3584]
                return tl.rearrange("p (sq xh c) -> p xh sq c",
                                    sq=4, xh=2)[:, h]

            ABC = {}
            for al in ('A', 'B', 'C'):
                ABC[al] = p2.tile([128, 3584], F16, tag="AL" + al, name="AL" + al)
            stg2 = {}
            for ygr in range(4):
                stg2[ygr] = p2.tile([64, 1792], F16, tag=f"st2_{ygr}", name=f"st2_{ygr}")

            pps_cm = tc.tile_pool(name="ps2s", bufs=2, space="PSUM")
            pps = pps_cm.__enter__()
            for gout in (0, 1):
                # ---- dx tensors + shifted tiles (h1-order, full g-block) ----
                def gtile(name):
                    return p2.tile([128, 3584], F16, tag=name, bufs=1,
                                   name=f"{name}_{gout}")
                dxp_g, dxm_g = gtile("dxp_g"), gtile("dxm_g")
                h1sq = h1.rearrange("p (sq g c) -> p sq g c", sq=4, g=2)
                dpv = dxp_g.rearrange("p (sq c) -> p sq c", sq=4)
                dmv = dxm_g.rearrange("p (sq c) -> p sq c", sq=4)
                V.tensor_sub(dpv[:, :, 0:832], h1sq[:, :, gout, 64:896],
                             h1sq[:, :, gout, 0:832])
                GP.memset(dpv[:, :, 832:896], 0.0)
                V.tensor_sub(dmv[:, :, 64:896], h1sq[:, :, gout, 0:832],
                             h1sq[:, :, gout, 64:896])
                GP.memset(dmv[:, :, 0:64], 0.0)
                sh2 = {}

                def h1g_src(p0, np_):
                    # strided g-block view of h1 rows [p0, p0+np_)
                    return AP(h1.tensor, h1.offset + p0 * 7168 + gout * 896,
                              [[7168, np_], [1792, 4], [1, 896]])

                for base, nm in ((None, 'H'), (dxp_g, 'dxp'), (dxm_g, 'dxm')):
                    for dy, sfx in ((1, 'p8'), (-1, 'm8')):
                        tgt = gtile(nm + sfx)
                        if base is None:
                            if dy == 1:
                                nc.sync.dma_start(tgt[0:120], h1g_src(8, 120))
                                nc.sync.dma_start(tgt[120:128], h1g_src(120, 8))
                            else:
                                nc.sync.dma_start(tgt[8:128], h1g_src(0, 120))
                                nc.sync.dma_start(tgt[0:8], h1g_src(0, 8))
                        else:
                            if dy == 1:
                                nc.sync.dma_start(tgt[0:120], base[8:128])
                                nc.sync.dma_start(tgt[120:128], base[120:128])
                            else:
                                nc.sync.dma_start(tgt[8:128], base[0:120])
                                nc.sync.dma_start(tgt[0:8], base[0:8])
                        sh2[(nm, sfx)] = tgt

                for h in (0, 1):
                    # ---- off2 matmuls (big psums) ----
                    mons = {}
                    for im in ('offy', 'offx'):
                        psum = pps.tile([128, 2048], F32, tag="big2",
                                        name=f"pso2_{im}_{gout}_{h}")
                        psv = psum.rearrange("p (x s) -> p x s", s=256)
                        banks = {}
                        for (im2, k, gout2, h2, dxt, gin, ref) in off2_ref:
                            if im2 != im or gout2 != gout or h2 != h:
                                continue
                            Lap = cref(ref)
                            xs, xis = off2_rhs_cols(k, h, dxt)
                            for xo_, xi_ in zip(xs, xis):
                                lx = xo_ - h * 7
                                rhs = h1v[:, :, gin, xi_]
                                banks.setdefault(lx // 2, []).append(
                                    ((lx, dxt, gin), psv[:, lx], Lap, rhs))
                        emit_banked(banks)
                        for sgn, nm in ((1.0, 'p'), (-1.0, 'm')):
                            mt = p2.tile([128, 1792], F16, tag=f"mon_{im}{nm}", bufs=2,
                                         name=f"mon_{im}{nm}_{gout}{h}")
                            # write in psum-stream order (lx, sq, s) into the
                            # sq-major tile: col = sq*448 + lx*64 + s
                            mtv = mt.rearrange("p (sq lx s) -> p lx sq s",
                                               sq=4, s=64)
                            SC.activation(mtv, psum[:, 0:1792], AF.Relu,
                                          scale=(1.0 if sgn > 0 else -1.0))
                            mons[(im, sgn)] = mt
                    upq, umq = mons[('offy', 1.0)], mons[('offy', -1.0)]
                    vpq, vmq = mons[('offx', 1.0)], mons[('offx', -1.0)]

                    # ---- deform2 (psum-order outputs) ----
                    def q2(name, nb=2):
                        return p2.tile([128, 1792], F16, tag=name, bufs=nb,
                                       name=f"{name}_{gout}{h}")
                    S0q, e1, e2, D2q = q2("S0q"), q2("e1q", 1), q2("e2q", 1), q2("D2q")
                    V.tensor_mul(e1, vpq, shviewH(dxp_g, h))
                    V.tensor_mul(e2, vmq, shviewH(dxm_g, h))
                    V.tensor_add(S0q, hviewH(gout, h), e1)
                    V.tensor_add(S0q, S0q, e2)
                    for sfx, mono in (('p8', upq), ('m8', umq)):
                        V.tensor_mul(e2, vpq, shviewH(sh2[('dxp', sfx)], h))
                        V.tensor_add(e1, shviewH(sh2[('H', sfx)], h), e2)
                        V.tensor_mul(e2, vmq, shviewH(sh2[('dxm', sfx)], h))
                        V.tensor_add(e1, e1, e2)
                        V.tensor_sub(e1, e1, S0q)
                        V.tensor_mul(e1, mono, e1)
                        if sfx == 'p8':
                            V.tensor_add(D2q, S0q, e1)
                        else:
                            V.tensor_add(D2q, D2q, e1)
                    # ---- ABC fill: sq-major (sq,lx,s) -> (sq,xi,s) ----
                    for al, y0 in (('A', 0), ('B', 2), ('C', 6)):
                        src = AP(D2q.tensor,
                                 D2q.offset + (y0 + 1) * 8 * 1792,
                                 [[1792, 64], [448, 4], [1, 448]])
                        dst = AP(ABC[al].tensor,
                                 ABC[al].offset + gout * 64 * 3584 + h * 448,
                                 [[3584, 64], [896, 4], [1, 448]])
                        nc.sync.dma_start(dst, src)

            pps_cm.__exit__(None, None, None)
            ppool2_cm = tc.tile_pool(name="ps2b", bufs=2, space="PSUM")
            ppool = ppool2_cm.__enter__()
            # ---- conv2 (N=512 pairs) + pool2 -> stg2; fc interleaved ----
            fcps = ppool.tile([128, 512], F32, tag="fcps", bufs=1, name="fcps")
            fc_first = True
            for ygr in range(4):
                for w0, w1_ in ((0, 4), (4, 7)):
                    nxp = w1_ - w0
                    psum = ppool.tile([128, 512 * nxp], F32,
                                      tag=("c2a" if w0 == 0 else "c2b"), bufs=1,
                                      name=f"psc2_{ygr}_{w0}")
                    banks = {}
                    for dx in range(-2, 3):
                        if (ygr, dx) not in c2_ref:
                            continue
                        al, ref = c2_ref[(ygr, dx)]
                        Lap = cref(ref)
                        for xp in range(w0, w1_):
                            valid = [j for j in (0, 1) if 0 <= 2 * xp + j + dx <= 13]
                            if not valid:
                                continue
                            j0, j1 = valid[0], valid[-1]
                            xi0 = 2 * xp + j0 + dx
                            nj = (j1 - j0 + 1) * 64
                            rhs = ABC[al].rearrange("p (sq c) -> p sq c",
                                                    sq=4)[:, :, xi0 * 64:xi0 * 64 + nj]
                            out = psum.rearrange("p (xp sq c) -> p xp sq c",
                                                 xp=nxp, sq=4)[:, xp - w0, :,
                                                               j0 * 64:j0 * 64 + nj]
                            banks.setdefault(xp - w0, []).append(
                                ((j0, dx), out, Lap, rhs))
                    emit_banked(banks)
                    pv = psum.rearrange("p (x sq two s) -> p x sq two s",
                                        x=nxp, sq=4, two=2, s=64)
                    pxa = spool.tile([128, 1024], F32, tag="pxa2", bufs=2,
                                     name=f"pxa2_{ygr}_{w0}")
                    pxav = pxa.rearrange("p (x s) -> p x s", s=256)
                    V.tensor_copy(pxav[:, 0:nxp], pv[:, 0:nxp, :, 0])
                    px = spool.tile([128, 1024], F16, tag="px2", name=f"px2_{ygr}_{w0}")
                    pxv = px.rearrange("p (x s) -> p x s", s=256)
                    V.tensor_max(pxv[:, 0:nxp], pxav[:, 0:nxp], pv[:, 0:nxp, :, 1])
                    pxs = spool.tile([64, 1024], F16, tag="pxs2", name=f"pxs2_{ygr}_{w0}")
                    nc.sync.dma_start(pxs[:, 0:nxp * 256], px[64:128, 0:nxp * 256])
                    py = spool.tile([64, 1024], F16, tag="py2", name=f"py2_{ygr}_{w0}")
                    V.tensor_max(py[:, 0:nxp * 256], px[0:64, 0:nxp * 256],
                                 pxs[:, 0:nxp * 256])
                    V.tensor_scalar(stg2[ygr][:, w0 * 256:w1_ * 256],
                                    py[:, 0:nxp * 256], b2c, 0.0, ALU.add, ALU.max)
                # fc matmuls for this ygr (hidden under later conv2 groups)
                for xq in range(7):
                    Lap = cref(fc_ref[(ygr, xq)])
                    rhs = stg2[ygr][:, xq * 256:(xq + 1) * 256]
                    MM(fcps[0:16, 0:256], Lap, rhs,
                       start=fc_first, stop=(ygr == 3 and xq == 6), **TP)
                    fc_first = False

            outs = p2.tile([16, 256], F32, tag="outs", name="outs")
            SC.activation(outs, fcps[0:16, 0:256], AF.Identity, bias=fcb)
            nc.sync.dma_start(OUT_d, outs)
            ppool2_cm.__exit__(None, None, None)

    nc.compile()
    return nc, consts_np


_cache = {}

def kernel(**inputs):
    x = np.asarray(inputs['x'], np.float32)          # (2048,1,28,28)
    w = {k: np.asarray(v) for k, v in inputs.items() if k != 'x'}
    if 'prog' not in _cache:
        _cache['prog'] = build_program(w)
    nc, consts = _cache['prog']
    in_maps = []
    for c in range(NCORES):
        m = dict(consts)
        m['XH'] = host_prep_x(x[c * S:(c + 1) * S]).astype(np.float16)
        in_maps.append(m)
    from concourse.bass_utils import run_bass_kernel_spmd
    res = run_bass_kernel_spmd(nc, in_maps, core_ids=list(range(NCORES)))
    outs = [r['OUT'][:10].T for r in res.results]
    return np.ascontiguousarray(np.concatenate(outs, 0).astype(np.float32))


# revision 28
# speedup vs baseline: 1.0126x; 1.0126x over previous
"""Trainium2 Bass kernel for nn_Net_Deform (deformable-conv CNN).

Self-contained: shards the batch over 8 NeuronCores (pure data parallel),
runs a Bass/Tile program per core, gathers full output.

v2 layout notes (S=256 samples/core, s = sq*64 + s64):
 F1 (XH): [128 p=(y+2)*4+sq, 2048 c=(x+2)*64+s64], y,x in [-2,30),
     replicate-padded on host, fp16.
 D1X: deformed1 with zero pads, same indexing as F1. fp16.
 conv1: single shared lhsT [120,128], K=(d5, ryi6, sq4) rel to y-pair t,
     via im2col IC_t [120, 1792] built with 5 DMAs/t from D1X.
     M = par*64 + sq*16 + co.
 h1 (F2): [128 p=(yy+1)*8+ci8, 7168 c=sq*1792 + (ci//8)*896 + xx*64 + s64]
     pad rows yy=-1/14 replicated. Scatter via DRAM bounce (2 DMAs/t).
 off2 psums/monos/deform2 tiles: psum-order cols lx*256 + (sq*64+s64).
 ABC (conv2 K relayout): [128 p = (ci//8)*64 + ai*8 + (ci%8), 3584 c =
     xi*256 + s], ai = yi - y0, bases A=0 B=2 C=6; filled with ONE plain
     64-partition DMA per (quarter, alloc).
 conv2 M = par*64 + cc*2 + yh; fc as baseline.
All lhsT consts packed into one [128, NC] fp16 dram tensor (1 DMA) +
one [64, 4] fp32 bias pack.
"""
import sys
sys.path.insert(0, '/opt/trn_rl_repo')
import numpy as np
from contextlib import ExitStack
import concourse.bass as bass
import concourse.mybir as mybir
import concourse.bacc as bacc
import concourse.tile as tile
from concourse.ap import AP

S = 256
SQ, S64 = 4, 64
NCORES = 8

def f1p(y, sq): return (y + 2) * 4 + sq
def f1c(x, s64=0): return (x + 2) * 64 + s64

_YY = np.clip(np.arange(-2, 30), 0, 27)
def host_prep_x(x_core):
    """(S,1,28,28) -> F1 [128, 2048] replicate-padded."""
    img = x_core.reshape(S, 28, 28)
    big = img[:, _YY][:, :, _YY]                      # (S, 32, 32)
    b = big.reshape(SQ, S64, 32, 32)
    out = b.transpose(2, 0, 3, 1).reshape(128, 2048)  # (y,sq,x,s64)
    return np.ascontiguousarray(out.astype(np.float32))

# ---------------- off1 schedule (F1 -> F1-order psum) ----------------
def off1_shuffle(y, x, k):
    j = 56 * y + 2 * x + k
    c = j // 784
    rem = j % 784
    return c, rem // 28, rem % 28

def build_off1_sched(off1_w):
    W = np.asarray(off1_w, np.float32)
    ent = []
    for k in (0, 1):
        for h in (0, 1):
            for dxt in (0, 1, 2):
                L = np.zeros((128, 128), np.float32)
                nz = False
                for y in range(28):
                    c, yo, xo0 = off1_shuffle(y, h * 14, k)
                    for sq in range(SQ):
                        m = f1p(y, sq)
                        for dyt in (0, 1, 2):
                            yi = yo + dyt - 1
                            if 0 <= yi < 28:
                                L[f1p(yi, sq), m] += W[c, 0, dyt, dxt]
                                nz = True
                if nz:
                    ent.append(('offy' if k == 0 else 'offx', k, h, dxt, L))
    return ent

def off1_rhs_cols(k, h, dxt):
    xs, xis = [], []
    for x in range(h * 14, h * 14 + 14):
        xi = 2 * x + k - 28 * h + dxt - 1
        if 0 <= xi < 28:
            xs.append(x); xis.append(xi)
    return xs, xis

# ---------------- conv1 shared lhsT (im2col K = (d, ryi, sq)) ------------
def build_conv1_lhsT_shared(conv1_w, ry_lo=-2, ry_hi=3):
    """zero out K rows with ryi outside [ry_lo, ry_hi] (edge pads)."""
    W = np.asarray(conv1_w, np.float32)
    L = np.zeros((120, 128), np.float32)
    for d in range(5):
        for ryi in range(-2, 4):
            if not (ry_lo <= ryi <= ry_hi):
                continue
            for sq in range(4):
                k = d * 24 + (ryi + 2) * 4 + sq
                for par in range(2):
                    dy = ryi - par
                    if not (-2 <= dy <= 2):
                        continue
                    for co in range(16):
                        L[k, par * 64 + sq * 16 + co] = W[co, 0, dy + 2, d]
    return L

# ---------------- off2 schedule ----------------
def off2_shuffle(c, y, x, k):
    j = 392 * c + 28 * y + 2 * x + k
    ch = j // 196
    rem = j % 196
    return ch, rem // 14, rem % 14

def f2p(yy, ci): return (yy + 1) * 8 + (ci % 8)

def build_off2_sched(off2_w):
    W = np.asarray(off2_w, np.float32)
    ent = []
    for k in (0, 1):
        for gout in (0, 1):
            for h in (0, 1):
                for dxt in (0, 1, 2):
                    for gin in (0, 1):
                        L = np.zeros((128, 128), np.float32)
                        nz = False
                        for cc in range(gout * 8, gout * 8 + 8):
                            for y in range(14):
                                ch, yo, xo = off2_shuffle(cc, y, h * 7, k)
                                for dyt in (0, 1, 2):
                                    yi = yo + dyt - 1
                                    if not (0 <= yi < 14):
                                        continue
                                    for ci in range(gin * 8, gin * 8 + 8):
                                        L[f2p(yi, ci), f2p(y, cc)] += W[ch, ci, dyt, dxt]
                                        nz = True
                        if nz:
                            ent.append(('offy' if k == 0 else 'offx', k, gout, h, dxt, gin, L))
    return ent

def off2_rhs_cols(k, h, dxt):
    xs, xis = [], []
    for x in range(h * 7, h * 7 + 7):
        xi = 2 * x + k - 14 * h + dxt - 1
        if 0 <= xi < 14:
            xs.append(x); xis.append(xi)
    return xs, xis

# ---------------- conv2 schedule (ABC new K order) ----------------
CONV2_BASE = {0: ('A', 0), 1: ('B', 2), 2: ('C', 6), 3: ('C', 6)}
def abc_row(ci, ai):
    return (ci // 8) * 64 + ai * 8 + (ci % 8)

def build_conv2_lhsT(conv2_w):
    """{(ygr, dx): (al, lhsT[128,128])}; K row = abc_row(ci, yi - y0)."""
    W = np.asarray(conv2_w, np.float32)
    out = {}
    for ygr in range(4):
        al, y0 = CONV2_BASE[ygr]
        for dx in range(-2, 3):
            L = np.zeros((128, 128), np.float32)
            nz = False
            for par in range(2):
                for yh in range(2):
                    yp = ygr * 4 + 2 * yh + par
                    if yp >= 14:
                        continue
                    for cc in range(32):
                        m = par * 64 + cc * 2 + yh
                        for dy in range(-2, 3):
                            yi = yp + dy
                            if not (0 <= yi < 14):
                                continue
                            ai = yi - y0
                            assert 0 <= ai < 8
                            for ci in range(16):
                                L[abc_row(ci, ai), m] += W[cc, ci, dy + 2, dx + 2]
                                nz = True
            if nz:
                out[(ygr, dx)] = (al, L)
    return out

def build_fc_lhsT(fc_w):
    W = np.asarray(fc_w, np.float32)
    out = {}
    for ygr in range(4):
        for xq in range(7):
            L = np.zeros((64, 16), np.float32)
            for cc in range(32):
                for yh in range(2):
                    ypp = ygr * 2 + yh
                    if ypp >= 7:
                        continue
                    L[cc * 2 + yh, :10] = W[:, cc * 49 + ypp * 7 + xq]
            out[(ygr, xq)] = L
    return out


F32, F16 = mybir.dt.float32, mybir.dt.float16
AF = mybir.ActivationFunctionType
ALU = mybir.AluOpType
TP = dict(tile_position=(0, 0))


class ConstPack:
    """Packs fp16 [rows<=128, cols] consts into one [128, NC] tensor."""
    def __init__(self):
        self.mats = []      # list of np arrays (rows, cols) fp16
        self.index = {}     # key -> (col0, rows, cols)
        self.cols = 0

    def add(self, arr):
        a = np.ascontiguousarray(np.asarray(arr, np.float16))
        key = (a.shape, a.tobytes())
        if key not in self.index:
            self.index[key] = (self.cols, a.shape[0], a.shape[1])
            self.mats.append(a)
            self.cols += a.shape[1]
        return self.index[key]

    def host_array(self):
        out = np.zeros((128, self.cols), np.float16)
        c = 0
        for a in self.mats:
            out[0:a.shape[0], c:c + a.shape[1]] = a
            c += a.shape[1]
        return out


def build_program(w):
    # ---------- host-side schedules ----------
    off1 = build_off1_sched(w['off1_w'])
    c1L = {t: build_conv1_lhsT_shared(
        w['conv1_w'],
        ry_lo=(0 if t == 0 else -2),
        ry_hi=(1 if t == 13 else 3)) for t in range(14)}
    off2 = build_off2_sched(w['off2_w'])
    c2 = build_conv2_lhsT(w['conv2_w'])
    fcL = build_fc_lhsT(w['fc_w'])

    packA = ConstPack()   # phase-1: off1 + conv1 (freed after phase 1)
    packB = ConstPack()   # phase-2: off2 + conv2 + fc (persistent)
    off1_ref = [(im, k, h, dxt, packA.add(L)) for (im, k, h, dxt, L) in off1]
    c1_ref = {t: packA.add(c1L[t]) for t in range(14)}
    off2_ref = [(im, k, go, h, dxt, gin, packB.add(L))
                for (im, k, go, h, dxt, gin, L) in off2]
    c2_ref = {key: (al, packB.add(L)) for key, (al, L) in c2.items()}
    fc_ref = {key: packB.add(L) for key, L in fcL.items()}

    bias = np.zeros((64, 4), np.float32)
    bias[:, 0] = np.asarray(w['conv1_b'], np.float32)[np.arange(64) % 16]
    bias[:, 1] = np.asarray(w['conv2_b'], np.float32)[np.arange(64) // 2]
    bias[0:10, 2] = np.asarray(w['fc_b'], np.float32)

    consts_np = {'CPA': packA.host_array(), 'CPB': packB.host_array(),
                 'BIA': bias}

    nc = bacc.Bacc("TRN2", target_bir_lowering=False, debug=False)
    XH_d = nc.dram_tensor("XH", [128, 2048], F16, kind="ExternalInput").ap()
    CPA_d = nc.dram_tensor("CPA", [128, packA.cols], F16, kind="ExternalInput").ap()
    CPB_d = nc.dram_tensor("CPB", [128, packB.cols], F16, kind="ExternalInput").ap()
    BIA_d = nc.dram_tensor("BIA", [64, 4], F32, kind="ExternalInput").ap()
    SCR_d = nc.dram_tensor("SCR", [14, 64, 896], F16, kind="Internal").ap()
    OUT_d = nc.dram_tensor("OUT", [16, 256], F32, kind="ExternalOutput").ap()

    with ExitStack() as ctx:
        tc = ctx.enter_context(tile.TileContext(nc))
        V, SC, GP = nc.vector, nc.scalar, nc.gpsimd
        MM = nc.tensor.matmul

        CPBs = nc.alloc_sbuf_tensor("CPBs", [128, packB.cols], F16).ap()
        nc.sync.dma_start(CPBs, CPB_d)
        BIAs = nc.alloc_sbuf_tensor("BIAs", [64, 4], F32).ap()
        nc.scalar.dma_start(BIAs, BIA_d)

        CPAs = [None]

        def crefA(ref):
            c0, rows, cols = ref
            return CPAs[0][0:rows, c0:c0 + cols]

        def cref(ref):
            c0, rows, cols = ref
            return CPBs[0:rows, c0:c0 + cols]

        b64 = BIAs[0:64, 0:1]
        b2c = BIAs[0:64, 1:2]
        fcb = BIAs[0:16, 2:3]

        def emit_banked(banks):
            for bk in sorted(banks):
                lst = sorted(banks[bk], key=lambda e: e[0])
                for i, (key, out, Lap, rhs) in enumerate(lst):
                    MM(out, Lap, rhs, start=(i == 0), stop=(i == len(lst) - 1), **TP)

        spool = ctx.enter_context(tc.tile_pool(name="stg", bufs=2))

        # persistent
        h1 = nc.alloc_sbuf_tensor("h1", [128, 7168], F16).ap()

        # ================= PHASE 1 =================
        with tc.tile_pool(name="ph1", bufs=1) as p1, \
             tc.tile_pool(name="ps1", bufs=2, space="PSUM") as ppool:
            XH = p1.tile([128, 2048], F16, tag="XH", name="XH")
            nc.sync.dma_start(XH, XH_d)
            CPAt = p1.tile([128, packA.cols], F16, tag="CPA", name="CPAt")
            CPAs[0] = CPAt
            csplit = min(1664, packA.cols)
            nc.sync.dma_start(CPAt[:, 0:csplit], CPA_d[:, 0:csplit])
            if packA.cols > csplit:
                nc.scalar.dma_start(CPAt[:, csplit:], CPA_d[:, csplit:])

            def img1v(name):
                return p1.tile([128, 1792], F16, tag=name, name=name)

            # ---- dx tensors + shifted operands first (independent of off1) ----
            dxp, dxm = img1v("dxp"), img1v("dxm")
            XV = XH[:, 128:1920]
            V.tensor_sub(dxp, XH[:, f1c(1):f1c(29)], XV)
            V.tensor_sub(dxm, XH[:, f1c(-1):f1c(27)], XV)
            shifted = {}
            for base, nm in ((XV, 'X'), (dxp, 'dxp'), (dxm, 'dxm')):
                for dy, sfx in ((1, 'p4'), (-1, 'm4')):
                    tgt = img1v(nm + sfx)
                    q = nc.sync
                    if dy == 1:
                        q.dma_start(tgt[0:124], base[4:128])
                        q.dma_start(tgt[124:128], base[124:128])
                    else:
                        q.dma_start(tgt[4:128], base[0:124])
                        q.dma_start(tgt[0:4], base[0:4])
                    shifted[(nm, sfx)] = tgt

            # ---- off1 matmuls ----
            ps_off = {}
            for im in ('offy', 'offx'):
                ps_off[im] = ppool.tile([128, 2048], F32, tag="big", name=f"ps_{im}")
            acc = {(im, x): [] for im in ('offy', 'offx') for x in range(28)}
            for (im, k, h, dxt, ref) in off1_ref:
                Lap = crefA(ref)
                xs, xis = off1_rhs_cols(k, h, dxt)
                runs = []
                for xo_, xi_ in zip(xs, xis):
                    if runs and runs[-1][-1][0] == xo_ - 1 and \
                       (xo_ % 8) != 0 and len(runs[-1]) < 8:
                        runs[-1].append((xo_, xi_))
                    else:
                        runs.append([(xo_, xi_)])
                for r in runs:
                    x0, xi0, n = r[0][0], r[0][1], len(r)
                    rhs = XH.rearrange("p (x s) -> p x s", s=64)[:, xi0 + 2:xi0 + 2 + 2 * n]
                    rhs = rhs.rearrange("p (n two) s -> p two n s", two=2)[:, 0]
                    acc[(im, x0)].append((ps_off[im].rearrange("p (x s) -> p x s", s=64)[:, x0:x0 + n],
                                          rhs, Lap))
            banks = {}
            for im in ('offy', 'offx'):
                for x in range(28):
                    for (out, rhs, Lap) in acc[(im, x)]:
                        banks.setdefault((im, x // 8), []).append(
                            ((x, -(x + out.shape[1])), out, Lap, rhs))
            emit_banked(banks)

            # ---- monos ----
            up1, um1 = img1v("up1"), img1v("um1")
            vp1, vm1 = img1v("vp1"), img1v("vm1")
            SC.activation(up1, ps_off['offy'][:, 0:1792], AF.Relu)
            V.tensor_scalar(um1, ps_off['offy'][:, 0:1792], -1.0, 0.0, ALU.mult, ALU.max)
            SC.activation(vp1, ps_off['offx'][:, 0:1792], AF.Relu)
            V.tensor_scalar(vm1, ps_off['offx'][:, 0:1792], -1.0, 0.0, ALU.mult, ALU.max)

            # ---- deform1 (all vector) ----
            S0, e1, e2, e3 = img1v("S0"), img1v("e1"), img1v("e2"), img1v("e3")
            D1X = p1.tile([128, 2048], F16, tag="D1X", name="D1X")
            GP.memset(D1X[:, 0:128], 0.0)
            GP.memset(D1X[:, 1920:2048], 0.0)
            V.tensor_mul(e1, vp1, dxp)
            V.tensor_mul(e2, vm1, dxm)
            V.tensor_add(S0, XV, e1)
            V.tensor_add(S0, S0, e2)
            for sfx, mono, dst in (('p4', up1, e1), ('m4', um1, e3)):
                V.tensor_mul(e2, vp1, shifted[('dxp', sfx)])
                V.tensor_add(dst, shifted[('X', sfx)], e2)
                V.tensor_mul(e2, vm1, shifted[('dxm', sfx)])
                V.tensor_add(dst, dst, e2)
                V.tensor_sub(dst, dst, S0)
                V.tensor_mul(dst, mono, dst)
            V.tensor_add(e1, e1, e3)
            V.tensor_add(D1X[:, 128:1920], S0, e1)

            # ---- conv1 via im2col ----
            icq = [nc.scalar, nc.gpsimd, nc.gpsimd, nc.scalar, nc.gpsimd]
            for t in range(14):
                C1 = crefA(c1_ref[t])
                IC = p1.tile([120, 1792], F16, tag="IC", bufs=4, name=f"IC_{t}")
                for d in range(5):
                    icq[(t * 5 + d) % 5].dma_start(
                        IC[d * 24:(d + 1) * 24],
                        D1X[8 * t:8 * t + 24, d * 64:d * 64 + 1792])
                psum = ppool.tile([128, 2048], F32, tag="big", name=f"psc1_{t}")
                for b in range(4):
                    ncol = 512 if b < 3 else 256
                    MM(psum[:, b * 512:b * 512 + ncol], C1,
                       IC[0:120, b * 512:b * 512 + ncol], start=True, stop=True, **TP)
                pv = psum.rearrange("p (xh two s) -> p xh two s", two=2, s=64)
                pxa = spool.tile([128, 896], F32, tag="pxa", bufs=2, name=f"pxa_{t}")
                V.tensor_copy(pxa.rearrange("p (x s) -> p x s", s=64), pv[:, 0:14, 0])
                px = spool.tile([128, 896], F16, tag="px", name=f"px_{t}")
                V.tensor_max(px.rearrange("p (x s) -> p x s", s=64),
                             pxa.rearrange("p (x s) -> p x s", s=64), pv[:, 0:14, 1])
                pxs = spool.tile([64, 896], F16, tag="pxs", name=f"pxs_{t}")
                nc.sync.dma_start(pxs, px[64:128])
                py = spool.tile([64, 896], F16, tag="py", name=f"py_{t}")
                V.tensor_max(py, px[0:64], pxs)
                stg = spool.tile([64, 896], F16, tag="stgc1", name=f"stg_{t}")
                V.tensor_scalar(stg, py, b64, 0.0, ALU.add, ALU.max)
                # bounce through DRAM, then ONE coalesced scatter into h1
                # h1 col = sq*1792 + gp*896 + x*64 + s; src (co, sq, gp, c):
                # sq/gp merge on src (14336 = 2*7168), (gp, c) stay split on dst
                nc.sync.dma_start(SCR_d[t], stg)
                dsts = [(t + 1) * 8]
                if t == 0:
                    dsts.append(0)
                if t == 13:
                    dsts.append(120)
                for p0 in dsts:
                    src = AP(SCR_d.tensor, SCR_d.offset + t * 64 * 896,
                             [[896, 8], [7168, 8], [1, 896]])
                    dst = AP(h1.tensor, h1.offset + p0 * 7168,
                             [[7168, 8], [896, 8], [1, 896]])
                    nc.sync.dma_start(dst, src)

        # ================= PHASE 2 =================
        with tc.tile_pool(name="ph2", bufs=1) as p2:
            h1v = h1.rearrange("p (sq g x s) -> p sq g x s", sq=4, g=2, s=64)

            def hviewH(g, h):
                # sq-major half view of h1 g-block: (sq: 1792, 4)(c: 448)
                return h1.rearrange("p (sq g xh c) -> p g xh sq c",
                                    sq=4, g=2, xh=2)[:, g, h]

            def shviewH(tl, h):
                # sq-major half view of a g-block tile [128, 3584]
                return tl.rearrange("p (sq xh c) -> p xh sq c",
                                    sq=4, xh=2)[:, h]

            ABC = {}
            for al in ('A', 'B', 'C'):
                ABC[al] = p2.tile([128, 3584], F16, tag="AL" + al, name="AL" + al)
            stg2 = {}
            for ygr in range(4):
                stg2[ygr] = p2.tile([64, 1792], F16, tag=f"st2_{ygr}", name=f"st2_{ygr}")

            pps_cm = tc.tile_pool(name="ps2s", bufs=2, space="PSUM")
            pps = pps_cm.__enter__()
            for gout in (0, 1):
                # ---- dx tensors + shifted tiles (h1-order, full g-block) ----
                def gtile(name):
                    return p2.tile([128, 3584], F16, tag=name, bufs=1,
                                   name=f"{name}_{gout}")
                dxp_g, dxm_g = gtile("dxp_g"), gtile("dxm_g")
                h1sq = h1.rearrange("p (sq g c) -> p sq g c", sq=4, g=2)
                dpv = dxp_g.rearrange("p (sq c) -> p sq c", sq=4)
                dmv = dxm_g.rearrange("p (sq c) -> p sq c", sq=4)
                V.tensor_sub(dpv[:, :, 0:832], h1sq[:, :, gout, 64:896],
                             h1sq[:, :, gout, 0:832])
                GP.memset(dpv[:, :, 832:896], 0.0)
                V.tensor_sub(dmv[:, :, 64:896], h1sq[:, :, gout, 0:832],
                             h1sq[:, :, gout, 64:896])
                GP.memset(dmv[:, :, 0:64], 0.0)
                sh2 = {}

                def h1g_src(p0, np_):
                    # strided g-block view of h1 rows [p0, p0+np_)
                    return AP(h1.tensor, h1.offset + p0 * 7168 + gout * 896,
                              [[7168, np_], [1792, 4], [1, 896]])

                for base, nm in ((None, 'H'), (dxp_g, 'dxp'), (dxm_g, 'dxm')):
                    for dy, sfx in ((1, 'p8'), (-1, 'm8')):
                        tgt = gtile(nm + sfx)
                        if base is None:
                            if dy == 1:
                                nc.sync.dma_start(tgt[0:120], h1g_src(8, 120))
                                nc.sync.dma_start(tgt[120:128], h1g_src(120, 8))
                            else:
                                nc.sync.dma_start(tgt[8:128], h1g_src(0, 120))
                                nc.sync.dma_start(tgt[0:8], h1g_src(0, 8))
                        else:
                            if dy == 1:
                                nc.sync.dma_start(tgt[0:120], base[8:128])
                                nc.sync.dma_start(tgt[120:128], base[120:128])
                            else:
                                nc.sync.dma_start(tgt[8:128], base[0:120])
                                nc.sync.dma_start(tgt[0:8], base[0:8])
                        sh2[(nm, sfx)] = tgt

                for h in (0, 1):
                    # ---- off2 matmuls (big psums) ----
                    mons = {}
                    for im in ('offy', 'offx'):
                        psum = pps.tile([128, 2048], F32, tag="big2",
                                        name=f"pso2_{im}_{gout}_{h}")
                        psv = psum.rearrange("p (x s) -> p x s", s=256)
                        banks = {}
                        for (im2, k, gout2, h2, dxt, gin, ref) in off2_ref:
                            if im2 != im or gout2 != gout or h2 != h:
                                continue
                            Lap = cref(ref)
                            xs, xis = off2_rhs_cols(k, h, dxt)
                            for xo_, xi_ in zip(xs, xis):
                                lx = xo_ - h * 7
                                rhs = h1v[:, :, gin, xi_]
                                banks.setdefault(lx // 2, []).append(
                                    ((lx, dxt, gin), psv[:, lx], Lap, rhs))
                        emit_banked(banks)
                        for sgn, nm in ((1.0, 'p'), (-1.0, 'm')):
                            mt = p2.tile([128, 1792], F16, tag=f"mon_{im}{nm}", bufs=2,
                                         name=f"mon_{im}{nm}_{gout}{h}")
                            # write in psum-stream order (lx, sq, s) into the
                            # sq-major tile: col = sq*448 + lx*64 + s
                            mtv = mt.rearrange("p (sq lx s) -> p lx sq s",
                                               sq=4, s=64)
                            SC.activation(mtv, psum[:, 0:1792], AF.Relu,
                                          scale=(1.0 if sgn > 0 else -1.0))
                            mons[(im, sgn)] = mt
                    upq, umq = mons[('offy', 1.0)], mons[('offy', -1.0)]
                    vpq, vmq = mons[('offx', 1.0)], mons[('offx', -1.0)]

                    # ---- deform2 (psum-order outputs) ----
                    def q2(name, nb=2):
                        return p2.tile([128, 1792], F16, tag=name, bufs=nb,
                                       name=f"{name}_{gout}{h}")
                    S0q, e1, e2, D2q = q2("S0q"), q2("e1q", 1), q2("e2q", 1), q2("D2q")
                    V.tensor_mul(e1, vpq, shviewH(dxp_g, h))
                    V.tensor_mul(e2, vmq, shviewH(dxm_g, h))
                    V.tensor_add(S0q, hviewH(gout, h), e1)
                    V.tensor_add(S0q, S0q, e2)
                    for sfx, mono in (('p8', upq), ('m8', umq)):
                        V.tensor_mul(e2, vpq, shviewH(sh2[('dxp', sfx)], h))
                        V.tensor_add(e1, shviewH(sh2[('H', sfx)], h), e2)
                        V.tensor_mul(e2, vmq, shviewH(sh2[('dxm', sfx)], h))
                        V.tensor_add(e1, e1, e2)
                        V.tensor_sub(e1, e1, S0q)
                        V.tensor_mul(e1, mono, e1)
                        if sfx == 'p8':
                            V.tensor_add(D2q, S0q, e1)
                        else:
                            V.tensor_add(D2q, D2q, e1)
                    # ---- ABC fill: sq-major (sq,lx,s) -> (sq,xi,s) ----
                    for al, y0 in (('A', 0), ('B', 2), ('C', 6)):
                        src = AP(D2q.tensor,
                                 D2q.offset + (y0 + 1) * 8 * 1792,
                                 [[1792, 64], [448, 4], [1, 448]])
                        dst = AP(ABC[al].tensor,
                                 ABC[al].offset + gout * 64 * 3584 + h * 448,
                                 [[3584, 64], [896, 4], [1, 448]])
                        nc.sync.dma_start(dst, src)

            pps_cm.__exit__(None, None, None)
            ppool2_cm = tc.tile_pool(name="ps2b", bufs=2, space="PSUM")
            ppool = ppool2_cm.__enter__()
            # ---- conv2 (N=512 pairs) + pool2 -> stg2; fc interleaved ----
            fcps = ppool.tile([128, 512], F32, tag="fcps", bufs=1, name="fcps")
            fc_first = True
            for ygr in range(4):
                for w0, w1_ in ((0, 4), (4, 7)):
                    nxp = w1_ - w0
                    psum = ppool.tile([128, 512 * nxp], F32,
                                      tag=("c2a" if w0 == 0 else "c2b"), bufs=1,
                                      name=f"psc2_{ygr}_{w0}")
                    banks = {}
                    for dx in range(-2, 3):
                        if (ygr, dx) not in c2_ref:
                            continue
                        al, ref = c2_ref[(ygr, dx)]
                        Lap = cref(ref)
                        for xp in range(w0, w1_):
                            valid = [j for j in (0, 1) if 0 <= 2 * xp + j + dx <= 13]
                            if not valid:
                                continue
                            j0, j1 = valid[0], valid[-1]
                            xi0 = 2 * xp + j0 + dx
                            nj = (j1 - j0 + 1) * 64
                            rhs = ABC[al].rearrange("p (sq c) -> p sq c",
                                                    sq=4)[:, :, xi0 * 64:xi0 * 64 + nj]
                            out = psum.rearrange("p (xp sq c) -> p xp sq c",
                                                 xp=nxp, sq=4)[:, xp - w0, :,
                                                               j0 * 64:j0 * 64 + nj]
                            banks.setdefault(xp - w0, []).append(
                                ((j0, dx), out, Lap, rhs))
                    emit_banked(banks)
                    pv = psum.rearrange("p (x sq two s) -> p x sq two s",
                                        x=nxp, sq=4, two=2, s=64)
                    pxa = spool.tile([128, 1024], F32, tag="pxa2", bufs=2,
                                     name=f"pxa2_{ygr}_{w0}")
                    pxav = pxa.rearrange("p (x s) -> p x s", s=256)
                    V.tensor_copy(pxav[:, 0:nxp], pv[:, 0:nxp, :, 0])
                    px = spool.tile([128, 1024], F16, tag="px2", name=f"px2_{ygr}_{w0}")
                    pxv = px.rearrange("p (x s) -> p x s", s=256)
                    V.tensor_max(pxv[:, 0:nxp], pxav[:, 0:nxp], pv[:, 0:nxp, :, 1])
                    pxs = spool.tile([64, 1024], F16, tag="pxs2", name=f"pxs2_{ygr}_{w0}")
                    nc.sync.dma_start(pxs[:, 0:nxp * 256], px[64:128, 0:nxp * 256])
                    py = spool.tile([64, 1024], F16, tag="py2", name=f"py2_{ygr}_{w0}")
                    V.tensor_max(py[:, 0:nxp * 256], px[0:64, 0:nxp * 256],
                                 pxs[:, 0:nxp * 256])
                    V.tensor_scalar(stg2[ygr][:, w0 * 256:w1_ * 256],
                                    py[:, 0:nxp * 256], b2c, 0.0, ALU.add, ALU.max)
                # fc matmuls for this ygr (hidden under later conv2 groups)
                for xq in range(7):
                    Lap = cref(fc_ref[(ygr, xq)])
                    rhs = stg2[ygr][:, xq * 256:(xq + 1) * 256]
                    MM(fcps[0:16, 0:256], Lap, rhs,
                       start=fc_first, stop=(ygr == 3 and xq == 6), **TP)
                    fc_first = False

            outs = p2.tile([16, 256], F32, tag="outs", name="outs")
            SC.activation(outs, fcps[0:16, 0:256], AF.Identity, bias=fcb)
            nc.sync.dma_start(OUT_d, outs)
            ppool2_cm.__exit__(None, None, None)

    nc.compile()
    return nc, consts_np


_cache = {}

def kernel(**inputs):
    x = np.asarray(inputs['x'], np.float32)          # (2048,1,28,28)
    w = {k: np.asarray(v) for k, v in inputs.items() if k != 'x'}
    if 'prog' not in _cache:
        _cache['prog'] = build_program(w)
    nc, consts = _cache['prog']
    in_maps = []
    for c in range(NCORES):
        m = dict(consts)
        m['XH'] = host_prep_x(x[c * S:(c + 1) * S]).astype(np.float16)
        in_maps.append(m)
    from concourse.bass_utils import run_bass_kernel_spmd
    res = run_bass_kernel_spmd(nc, in_maps, core_ids=list(range(NCORES)))
    outs = [r['OUT'][:10].T for r in res.results]
    return np.ascontiguousarray(np.concatenate(outs, 0).astype(np.float32))
